# revision 1
# baseline (speedup 1.0000x reference)
"""Trainium2 Bass kernel for nn_EW_MHSA_Hybrid (hybrid window MHSA).

Reference computation (per image, C=256, H=W=56, WS=7, 4 heads x 64 dim):
  qk = conv1x1(x, qk_w)            # 512 channels = [q(4x64) | k(4x64)]
  v  = relu(conv1x1(x, v_w))       # 256 channels
  for each partition type (close 7x7 blocks, remote 8-dilated 7x7 grids):
      per 49-pixel window, per head: softmax((q k^T) / 8) @ v
  out = close_result + remote_result

Sharding: data-parallel over batch B=32 across 8 cores (4 images/core),
weights replicated.

Device-side design (per core):
  - The host supplies x twice, pixel-permuted to window-major order for the
    close and remote partitionings, so every window is a contiguous run of
    49 pixels.  (Walrus requires the matmul stationary operand to have a
    single free dimension, so windows must be contiguous.)
  - 1x1 convs are plain matmuls; conv outputs stay in SBUF (bf16),
    window-major.
  - Scores are computed transposed: psum[keys, queries] = k^T q, so the exp
    output directly serves as the stationary operand of the second matmul
    (contraction over keys) -- no transposes anywhere.
  - Windows are processed in pairs: even window at partition base 0, odd at
    base 64 (matmul operand partition bases must be 0/64 when K<=64), so
    exp / relu / normalize cover ~113 partitions per instruction.
  - v^T (pixels x channels) is produced directly by a conv whose stationary
    operand is the window pixel slice of x; a ones column appended per head
    makes the second attention matmul also emit the softmax denominator.
  - Output is written pixel-major [px, ch], window-major px order, batched
    into few large DMAs; the host un-permutes, sums the two partition types
    and transposes back to [C, H, W].
"""

import sys

sys.path.insert(0, "/opt/trn_rl_repo")
sys.path.insert(0, "/opt/pypackages")

import numpy as np
import ml_dtypes

import concourse.bass as bass
import concourse.mybir as mybir
import concourse.tile as tile
from concourse.bass_utils import run_bass_kernel_spmd

F32 = mybir.dt.float32
BF16 = mybir.dt.bfloat16
F8 = mybir.dt.float8e4

N_CORES = 8
B_PER_CORE = 4
C = 256
H = W = 56
HW = H * W  # 3136
WS = 7
NW = 49  # window pixels
NWIN = 64  # windows per image per partition type
NPAIR = NWIN // 2
PAIR_GRP = 8  # pairs per output-accumulation tile / DMA batch
HEADS = 4
DH = 64
SCALE = DH ** -0.5
PXC = 448  # conv pixel-chunk (divides 3136 into 7)


def _perms():
    """close/remote window-major pixel permutation (window-major -> raster)."""
    close = np.empty(HW, np.int64)
    remote = np.empty(HW, np.int64)
    i = 0
    for wi in range(8):
        for wj in range(8):
            for r in range(WS):
                for c in range(WS):
                    close[i] = (7 * wi + r) * 56 + 7 * wj + c
                    remote[i] = (8 * r + wi) * 56 + 8 * c + wj
                    i += 1
    return close, remote


CLOSE_PERM, REMOTE_PERM = _perms()


def split_multi_waits(nc, donate=True):
    """This walrus build supports at most 1 sync-wait per instruction.

    Hoist extra waits onto the nearest preceding same-engine instruction with
    a free wait slot (usually the matmul's own Ldweights).  Those waits park
    in the engine's wait queue instead of blocking the sequencer, which a
    NoOp-with-wait would do (a seq-only instruction holds the SEQ while its
    wait is pending, serializing all later dispatch on that engine).  Only
    when no such slot exists do we fall back to inserting a NoOp."""
    for fn in nc.m.functions:
        for blk in fn.blocks:
            insts = blk.instructions
            k = 0
            while k < len(insts):
                inst = insts[k]
                si = inst.sync_info
                if si is not None and len(si.on_wait) > 1:
                    waits = list(si.on_wait)
                    extra = waits[:-1]
                    # donate waits to the contiguous run of sync-free
                    # same-engine instructions directly before (in practice
                    # the matmul's Ldweights).  Moving a wait earlier on the
                    # same in-order engine is conservative; stopping at the
                    # first instruction with any sync_info avoids reordering
                    # a wait ahead of a sem update its satisfier needs.
                    j = k - 1
                    while donate and extra and j >= 0:
                        pj = insts[j]
                        if pj.engine != inst.engine:
                            j -= 1
                            continue
                        sj = pj.sync_info
                        if (
                            isinstance(pj, (mybir.InstEventSemaphore, mybir.InstNoOp))
                            or (sj is not None and (sj.on_wait or sj.on_update))
                        ):
                            break
                        pj.sync_info = mybir.SyncInfo(
                            on_wait=[extra.pop()], on_update=[]
                        )
                        j -= 1
                    for w in extra:
                        nop = mybir.InstNoOp(
                            name=nc.get_next_instruction_name(), ins=[], outs=[]
                        )
                        nop.engine = inst.engine
                        nop.sync_info = mybir.SyncInfo(on_wait=[w], on_update=[])
                        nc.register_instruction(nop, overwrite=True)
                        insts.insert(k, nop)
                        k += 1
                    inst.sync_info = mybir.SyncInfo(
                        on_wait=[waits[-1]], on_update=list(si.on_update)
                    )
                k += 1


def build_nc(gap_ok=True, repeat=1, n_imgs=B_PER_CORE, hw_loop=1):
    nc = bass.Bass("TRN2")

    x_d = [
        nc.declare_dram_parameter(f"x{pt}", [n_imgs, C, HW], F32, isOutput=False)
        for pt in range(2)
    ]
    qkw_d = nc.declare_dram_parameter("qkw", [2, 128, 512], BF16, isOutput=False)
    x8_d = [
        nc.declare_dram_parameter(f"x8{pt}", [n_imgs, 2, 128, HW], F8, isOutput=False)
        for pt in range(2)
    ]
    kw8_d = nc.declare_dram_parameter("kw8", [128, 2, 256], F8, isOutput=False)
    vw_d = nc.declare_dram_parameter("vw", [2, 128, 256], BF16, isOutput=False)
    out_d = nc.declare_dram_parameter(
        "out", [2, n_imgs, HW, HEADS * 65], BF16, isOutput=True
    )

    with tile.TileContext(nc) as tc:
        with (
            tc.tile_pool(name="wpool", bufs=1) as wpool,
            tc.tile_pool(name="xpool", bufs=3) as xpool,
            tc.tile_pool(name="qkpool", bufs=2) as qkpool,
            tc.tile_pool(name="vtpool", bufs=3) as vtpool,
            tc.tile_pool(name="expool", bufs=3) as expool,
            tc.tile_pool(name="opool", bufs=2) as opool,
            tc.tile_pool(name="cps_pool", bufs=2, space="PSUM") as cps_pool,
            tc.tile_pool(name="vps_pool", bufs=2, space="PSUM") as vps_pool,
            tc.tile_pool(name="sps_pool", bufs=2, space="PSUM") as sps_pool,
            tc.tile_pool(name="ops_pool", bufs=2, space="PSUM") as ops_pool,
        ):
            wq = wpool.tile([128, 2, 512], BF16)
            nc.sync.dma_start(out=wq[:], in_=qkw_d.rearrange("k p o -> p k o"))
            wv = wpool.tile([128, 2, 256], BF16)
            nc.sync.dma_start(out=wv[:], in_=vw_d.rearrange("k p o -> p k o"))
            wk8 = wpool.tile([128, 2, 256], F8)
            nc.sync.dma_start(out=wk8[:], in_=kw8_d.rearrange("p k o -> p k o"))

            import contextlib

            loop_ctx = (
                tc.For_i(0, hw_loop, 1) if hw_loop > 1 else contextlib.nullcontext()
            )
            with loop_ctx:
                for _rep in range(repeat):
                    for img in range(n_imgs):
                        for ptype in range(2):
                            # ---- load x (cast fp32 -> bf16 in the DMA) ----
                            xt = xpool.tile([128, 2, HW], BF16, name="xt", tag="xt")
                            nc.gpsimd.dma_start(
                                out=xt[:],
                                in_=x_d[ptype][img].rearrange("(k p) n -> p k n", p=128),
                            )
                            x8t = xpool.tile([128, 2, HW], F8, name="x8t", tag="x8t")
                            nc.sync.dma_start(
                                out=x8t[:],
                                in_=x8_d[ptype][img].rearrange("k p n -> p k n"),
                            )

                            # ---- qk conv: 4 chunks of 128 out-channels ----
                            # qk[0]=q h01, qk[1]=q h23, qk[2]=k h01, qk[3]=k h23
                            qk = [
                                qkpool.tile([128, HW], BF16, name=f"qk{m}", tag=f"qk{m}")
                                for m in range(4)
                            ]
                            for m in range(4):
                                for p in range(HW // PXC):
                                    ps_full = cps_pool.tile(
                                        [128, 512], F32, name="ps", tag="ps"
                                    )
                                    ps = ps_full[:, :PXC]
                                    if m < 2:
                                        for kk in range(2):
                                            nc.tensor.matmul(
                                                ps[:],
                                                lhsT=wq[:, kk, m * 128 : (m + 1) * 128],
                                                rhs=xt[:, kk, p * PXC : (p + 1) * PXC],
                                                start=(kk == 0),
                                                stop=(kk == 1),
                                            )
                                    else:
                                        nc.tensor.matmul(
                                            ps[:],
                                            lhsT=wk8[
                                                :, :, (m - 2) * 128 : (m - 1) * 128
                                            ],
                                            rhs=x8t[:, :, p * PXC : (p + 1) * PXC],
                                            start=True,
                                            stop=True,
                                            perf_mode=mybir.MatmulPerfMode.DoubleRow,
                                        )
                                    nc.scalar.copy(
                                        out=qk[m][:, p * PXC : (p + 1) * PXC], in_=ps[:]
                                    )

                            # odd heads must sit at partition base 0 for matmul
                            # operands: shift rows 64-127 down via SBUF->SBUF DMA
                            qks = [
                                qkpool.tile([64, HW], BF16, name=f"qks{m}", tag=f"qks{m}")
                                for m in range(4)
                            ]
                            for m in range(4):
                                nc.sync.dma_start(out=qks[m][:], in_=qk[m][64:128, :])

                            for grp in range(NPAIR // PAIR_GRP):
                                o_acc = opool.tile(
                                    [128, PAIR_GRP, HEADS, 65], BF16,
                                    name="o_acc", tag="o_acc",
                                )
                                for pg in range(PAIR_GRP):
                                    pair = grp * PAIR_GRP + pg
                                    w0 = pair * 2

                                    # ---- v^T conv: [49, 2, 256] col-packed ----
                                    vt_ps_full = vps_pool.tile(
                                        [128, 512], F32, name="vt_ps", tag="vt_ps"
                                    )
                                    vt_ps = vt_ps_full[:NW].rearrange(
                                        "p (e d) -> p e d", e=2
                                    )
                                    mm = 0
                                    for e in range(2):
                                        px0 = (w0 + e) * NW
                                        for kk in range(2):
                                            nc.tensor.matmul(
                                                vt_ps[:, e, :],
                                                lhsT=xt[:, kk, px0 : px0 + NW],
                                                rhs=wv[:, kk, :],
                                                start=(mm == 0),
                                                stop=(mm == 3),
                                            )
                                            mm += 1

                                    # relu + per-head 65-col layout (+ones) bf16
                                    vt = vtpool.tile([NW, 2, HEADS, 65], BF16)
                                    nc.vector.tensor_scalar_max(
                                        out=vt[:, :, :, 0:64],
                                        in0=vt_ps.rearrange(
                                            "p e (h d) -> p e h d", h=HEADS
                                        ),
                                        scalar1=0.0,
                                    )
                                    nc.vector.memset(vt[:, :, :, 64:65], 1.0)

                                    # ---- scores^T: psum[keys, e, h, queries] ----
                                    sc_ps_full = sps_pool.tile(
                                        [128, 512], F32, name="sc_ps", tag="sc_ps"
                                    )
                                    sc_ps = sc_ps_full[:NW, : 2 * HEADS * NW].rearrange(
                                        "p (e h q) -> p e h q", e=2, h=HEADS
                                    )
                                    mm = 0
                                    for e in range(2):
                                        px0 = (w0 + e) * NW
                                        for h in range(4):
                                            if h % 2 == 0:
                                                kt = qk[2 + h // 2][0:64]
                                                qt = qk[h // 2][0:64]
                                            else:
                                                kt = qks[2 + h // 2][0:64]
                                                qt = qks[h // 2][0:64]
                                            nc.tensor.matmul(
                                                sc_ps[:, e, h, :],
                                                lhsT=kt[:, px0 : px0 + NW],
                                                rhs=qt[:, px0 : px0 + NW],
                                                start=True,
                                                stop=True,
                                            )
                                            mm += 1

                                    # ---- exp (scores are small: skip max-sub) ----
                                    ex = expool.tile([NW, 2, HEADS, NW], BF16)
                                    nc.scalar.activation(
                                        out=ex[:],
                                        in_=sc_ps[:],
                                        func=mybir.ActivationFunctionType.Exp,
                                        scale=SCALE / 16.0,
                                    )

                                    # ---- o~ = exp^T.T @ [v | 1]: [113, h, 65] ----
                                    # (partition-packed: only DVE reads it)
                                    o_ps_full = ops_pool.tile(
                                        [128, 512], F32, name="o_ps", tag="o_ps"
                                    )
                                    o_ps = o_ps_full[:113, : HEADS * 65].rearrange(
                                        "p (h v) -> p h v", h=HEADS
                                    )
                                    for e in range(2):
                                        b0 = 64 * e
                                        for h in range(4):
                                            nc.tensor.matmul(
                                                o_ps[b0 : b0 + NW, h, :],
                                                lhsT=ex[:, e, h, :],
                                                rhs=vt[:, e, h, :],
                                                start=(h == 0),
                                                stop=(h == 3),
                                            )

                                    # ---- copy o~ and sumexp out (host divides)
                                    if gap_ok:
                                        nc.vector.tensor_copy(
                                            out=o_acc[:113, pg], in_=o_ps[:]
                                        )
                                    else:
                                        for e in range(2):
                                            b0 = 64 * e
                                            nc.vector.tensor_copy(
                                                out=o_acc[b0 : b0 + NW, pg],
                                                in_=o_ps[b0 : b0 + NW],
                                            )

                                # ---- batched output DMA: evens then odds ----
                                # out px (window-major) for pair p, parity e:
                                #   (2p + e) * 49 + n
                                ob = out_d[ptype, img].rearrange(
                                    "(q e n) ch -> q e n ch", e=2, n=NW
                                )
                                for half in range(2):
                                    b0 = 64 * half
                                    nc.sync.dma_start(
                                        out=ob[
                                            grp * PAIR_GRP : (grp + 1) * PAIR_GRP,
                                            half,
                                        ].rearrange("q n ch -> n q ch"),
                                        in_=o_acc[b0 : b0 + NW],
                                    )

    split_multi_waits(nc, donate=True)
    return nc


_NC_CACHE = {}


def _get_nc(gap_ok=True, repeat=1):
    key = (gap_ok, repeat)
    if key not in _NC_CACHE:
        _NC_CACHE[key] = build_nc(gap_ok=gap_ok, repeat=repeat)
    return _NC_CACHE[key]


def _prep_in_maps(x, qk_w, v_w):
    xs = np.ascontiguousarray(
        x.reshape(N_CORES, B_PER_CORE, C, HW), dtype=np.float32
    )
    xc = np.ascontiguousarray(xs[:, :, :, CLOSE_PERM])
    xr = np.ascontiguousarray(xs[:, :, :, REMOTE_PERM])
    xc8 = np.ascontiguousarray(
        xc.reshape(N_CORES, B_PER_CORE, 2, 128, HW).astype(ml_dtypes.float8_e4m3fn)
    )
    xr8 = np.ascontiguousarray(
        xr.reshape(N_CORES, B_PER_CORE, 2, 128, HW).astype(ml_dtypes.float8_e4m3fn)
    )
    qkw = np.ascontiguousarray(qk_w.T.reshape(2, 128, 512).astype(ml_dtypes.bfloat16))
    kw8 = np.ascontiguousarray(
        (qk_w.T[:, 256:] * 16.0)
        .reshape(2, 128, 256)
        .transpose(1, 0, 2)
        .astype(ml_dtypes.float8_e4m3fn)
    )
    vw = np.ascontiguousarray(v_w.T.reshape(2, 128, 256).astype(ml_dtypes.bfloat16))
    return [
        {
            "x0": xc[c], "x1": xr[c], "x80": xc8[c], "x81": xr8[c],
            "qkw": qkw, "kw8": kw8, "vw": vw,
        }
        for c in range(N_CORES)
    ]


def _assemble(results):
    outs = []
    for c in range(N_CORES):
        o = np.asarray(results[c]["out"], dtype=np.float32)
        o = o.reshape(2, B_PER_CORE, HW, HEADS, 65)
        o = o[..., :64] / o[..., 64:65]  # softmax denominator
        o = o.reshape(2, B_PER_CORE, HW, C)
        full = np.empty((B_PER_CORE, HW, C), np.float32)
        full[:, CLOSE_PERM] = o[0]
        tmp = np.empty((B_PER_CORE, HW, C), np.float32)
        tmp[:, REMOTE_PERM] = o[1]
        full += tmp
        outs.append(full.transpose(0, 2, 1).reshape(B_PER_CORE, C, H, W))
    return np.ascontiguousarray(np.concatenate(outs, axis=0), dtype=np.float32)


def kernel(x, qk_w, v_w):
    nc = _get_nc()
    in_maps = _prep_in_maps(np.asarray(x), np.asarray(qk_w), np.asarray(v_w))
    res = run_bass_kernel_spmd(nc, in_maps, core_ids=list(range(N_CORES)))
    return _assemble(res.results)

